# revision 78
# baseline (speedup 1.0000x reference)
import numpy as np
import ml_dtypes
import concourse.bass as bass
import concourse.mybir as mybir
import concourse.tile as tile
from concourse import bacc
from concourse.bass_utils import run_bass_kernel_spmd
from concourse.alu_op_type import AluOpType

B, S, D = 4, 2048, 768
HPC = 6            # heads per core
PAIRS = 3
THETA = 10000.0
N_CORES = 8
F32 = mybir.dt.float32
BF16 = mybir.dt.bfloat16
BF = ml_dtypes.bfloat16
VW = HPC * 65      # 390: per-tb V tile width (6 heads x (64 dims + ones col))
EXP = mybir.ActivationFunctionType.Exp

_NC = None


def build_nc(with_collective=True):
    nc = bacc.Bacc("TRN2", target_bir_lowering=False, debug=False,
                   num_devices=N_CORES)
    xd = nc.dram_tensor("xd", [128, 4 * 3072], BF16, kind="ExternalInput")
    wqd = nc.dram_tensor("wqd", [128, 2304], BF16, kind="ExternalInput")
    wkd = nc.dram_tensor("wkd", [128, 2304], BF16, kind="ExternalInput")
    wvd = nc.dram_tensor("wvd", [128, 6 * VW], BF16, kind="ExternalInput")
    wod = nc.dram_tensor("wod", [128, 2304], BF16, kind="ExternalInput")
    cosd = nc.dram_tensor("cos", [128, S], BF16, kind="ExternalInput")
    sind = nc.dram_tensor("sin", [128, S], BF16, kind="ExternalInput")
    maskd = nc.dram_tensor("mask", [128, 128], BF16, kind="ExternalInput")
    eyed = nc.dram_tensor("eye", [128, 128], BF16, kind="ExternalInput")
    out = nc.dram_tensor("out", [S, D], F32, kind="ExternalOutput")

    with tile.TileContext(nc) as tc:
        with tc.tile_pool(name="persist", bufs=1) as pp, \
             tc.tile_pool(name="dram", bufs=1, space="DRAM") as dpool, \
             tc.tile_pool(name="uhp", bufs=3) as uhp, \
             tc.tile_pool(name="swp", bufs=3) as swp, \
             tc.tile_pool(name="etp", bufs=14) as etp, \
             tc.tile_pool(name="ctxtp", bufs=10) as ctxtp, \
             tc.tile_pool(name="rcpp", bufs=6) as rcpp, \
             tc.tile_pool(name="obp", bufs=2) as obp, \
             tc.tile_pool(name="pgen", bufs=3, space="PSUM") as pgen, \
             tc.tile_pool(name="pwide", bufs=2, space="PSUM") as pwide, \
             tc.tile_pool(name="pctx", bufs=1, space="PSUM") as pctxp:

            sb_q = [pp.tile([128, S], BF16, name=f"sb_q{i}") for i in range(PAIRS)]
            sb_k = [pp.tile([128, S], BF16, name=f"sb_k{i}") for i in range(PAIRS)]
            sb_v = pp.tile([128, 16 * VW], BF16)
            sb_ctx = [pp.tile([128, S], BF16, name=f"sb_ctx{i}") for i in range(PAIRS)]
            xcol = [pp.tile([128, 3072], BF16, name=f"xcol{i}") for i in range(4)]
            sb_wq = pp.tile([128, 2304], BF16)
            sb_wk = pp.tile([128, 2304], BF16)
            sb_wv = pp.tile([128, 6 * VW], BF16)
            sb_wo = pp.tile([128, 2304], BF16)
            sb_cos = pp.tile([128, S], BF16)
            sb_sin = pp.tile([128, S], BF16)
            sb_mask = pp.tile([128, 128], BF16)
            sb_eye = pp.tile([128, 128], BF16)
            bounce_in = dpool.tile([S, D], F32)
            bounce_out = dpool.tile([S, D], F32)

            def gen_tile():
                return pgen.tile([128, 512], F32, name="pgen_t")

            def wide_tile():
                return pwide.tile([128, 1024], F32, name="pw_t")

            # input loads, most-urgent first
            nc.sync.dma_start(xcol[0][:], xd[:, 0:3072])
            nc.sync.dma_start(sb_wk[:], wkd[:])
            nc.sync.dma_start(sb_cos[:, 0:512], cosd[:, 0:512])
            nc.sync.dma_start(sb_sin[:, 0:512], sind[:, 0:512])
            nc.sync.dma_start(sb_wq[:], wqd[:])
            nc.sync.dma_start(sb_wv[:], wvd[:])
            nc.sync.dma_start(sb_cos[:, 512:2048], cosd[:, 512:2048])
            nc.sync.dma_start(sb_sin[:, 512:2048], sind[:, 512:2048])
            nc.sync.dma_start(sb_mask[:], maskd[:])
            nc.sync.dma_start(sb_eye[:], eyed[:])
            # warm up the PE p-state before real work arrives
            warm = pp.tile([128, 512], BF16)
            nc.vector.memset(warm[:], 1.0)
            wt = gen_tile()
            for i in range(7):
                nc.tensor.matmul(wt[:], warm[:, 0:128], warm[:],
                                 start=(i == 0), stop=(i == 6))
            with nc.allow_low_precision(reason="warmup drain"):
                nc.vector.tensor_copy(warm[:], wt[:])

            def proj_tt(tt):
                """QKV projection + RoPE for token block tt (512 tokens).

                K pairs first, swap+add per half so attention unblocks early;
                V chains interleaved to keep PE busy while DVE drains rope.
                """
                xc = xcol[tt]
                csl = sb_cos[:, tt * 512:(tt + 1) * 512]
                ssl = sb_sin[:, tt * 512:(tt + 1) * 512]
                uh = uhp.tile([128, 3072], BF16)
                swf = swp.tile([128, 3072], BF16)

                def qk_pair(wi, wsb, dst, pr):
                    if tt == 0:
                        pcw = wide_tile()
                        pc = pcw[:, 0:512]
                    else:
                        pc = gen_tile()
                    for ck in range(6):
                        nc.tensor.matmul(
                            pc[:],
                            wsb[:, ck * 384 + pr * 128:
                                   ck * 384 + (pr + 1) * 128],
                            xc[:, ck * 512:(ck + 1) * 512],
                            start=(ck == 0), stop=(ck == 5))
                    with nc.allow_low_precision(reason="bf16 qk"):
                        nc.vector.tensor_mul(
                            dst[pr][:, tt * 512:(tt + 1) * 512], pc[:], csl)
                        nc.vector.tensor_mul(
                            uh[:, (wi * 3 + pr) * 512:(wi * 3 + pr + 1) * 512],
                            pc[:], ssl)

                def v_block(tj):
                    tb = tt * 4 + tj
                    pvt = gen_tile()
                    for ck in range(6):
                        nc.tensor.matmul(
                            pvt[:, 0:VW],
                            xc[:, ck * 512 + tj * 128:
                                  ck * 512 + tj * 128 + 128],
                            sb_wv[:, ck * VW:(ck + 1) * VW],
                            start=(ck == 0), stop=(ck == 5))
                    with nc.allow_low_precision(reason="bf16 v"):
                        nc.scalar.copy(sb_v[:, tb * VW:(tb + 1) * VW],
                                       pvt[:, 0:VW])
                    nc.vector.memset(sb_v[:, tb * VW + 64:(tb + 1) * VW:65], 1.0)

                def swap_add(wi, dst, pr):
                    # rope pair swap: within each 32-partition block the
                    # even/odd halves are interleaved at 16, so the swap is
                    # an in-block shuffle i^16 on the DVE crossbar
                    sl = slice((wi * 3 + pr) * 512, (wi * 3 + pr + 1) * 512)
                    nc.vector.stream_shuffle(
                        swf[:, sl], uh[:, sl], [i ^ 16 for i in range(32)])
                    d = dst[pr][:, tt * 512:(tt + 1) * 512]
                    # pair 0 gates the next q-block's first score: keep its
                    # add on the low-latency DVE; offload the rest to Pool
                    aeng = nc.vector if pr == 0 else nc.gpsimd
                    with nc.allow_low_precision(reason="bf16 qk add"):
                        aeng.tensor_add(d, d, swf[:, sl])

                for pr in range(PAIRS):
                    qk_pair(0, sb_wk, sb_k, pr)
                    swap_add(0, sb_k, pr)
                    yield
                    qk_pair(1, sb_wq, sb_q, pr)
                    swap_add(1, sb_q, pr)
                    yield
                for tj in range(4):
                    v_block(tj)
                    yield

            def finish_tb(qt, qj, ctxt):
                """Transpose ctx to [hd, tok], output projection, store."""
                tb = qt * 4 + qj
                for c in range(PAIRS):
                    ptr = gen_tile()
                    nc.tensor.matmul(
                        ptr[:, 0:128],
                        ctxt[:, c * 128:(c + 1) * 128],
                        sb_eye[:],
                        start=True, stop=True)
                    with nc.allow_low_precision(reason="bf16 ctxT"):
                        if qt == 3 and c % 2 == 0:
                            nc.scalar.copy(
                                sb_ctx[c][:, tb * 128:(tb + 1) * 128],
                                ptr[:, 0:128])
                        else:
                            nc.vector.tensor_copy(
                                sb_ctx[c][:, tb * 128:(tb + 1) * 128],
                                ptr[:, 0:128])
                ob = obp.tile([128, D], F32)
                for nn in range(2):
                    pot = gen_tile()
                    for ci in range(PAIRS):
                        nc.tensor.matmul(
                            pot[:, 0:384],
                            sb_ctx[ci][:, tb * 128:(tb + 1) * 128],
                            sb_wo[:, ci * 768 + nn * 384:
                                     ci * 768 + nn * 384 + 384],
                            start=(ci == 0), stop=(ci == 2))
                    if qt == 3 and nn == 0:
                        nc.scalar.copy(ob[:, nn * 384:(nn + 1) * 384],
                                       pot[:, 0:384])
                    else:
                        nc.vector.tensor_copy(ob[:, nn * 384:(nn + 1) * 384],
                                              pot[:, 0:384])
                if with_collective:
                    nc.gpsimd.dma_start(bounce_in[tb * 128:(tb + 1) * 128, :],
                                        ob[:])
                else:
                    nc.sync.dma_start(out[tb * 128:(tb + 1) * 128, :], ob[:])

            def attn_qt(qt):
                """Full causal attention for q-block qt (512 queries), all
                6 heads, then output projection for its 4 token blocks."""
                nkb = 4 * qt + 4
                ctxts = [ctxtp.tile([128, 384], BF16, name="ctq")
                         for _ in range(4)]
                for h in range(HPC):
                    pr, off = h // 2, (h % 2) * 64
                    pctx = pctxp.tile([128, 260], F32, name="pctx_t")
                    fulls = 4 * qt

                    def full_score(kb2):
                        psw = wide_tile()
                        for s in (0, 1):
                            kb = kb2 + s
                            nc.tensor.matmul(
                                psw[:, s * 512:(s + 1) * 512],
                                sb_k[pr][off:off + 64, kb * 128:(kb + 1) * 128],
                                sb_q[pr][off:off + 64, qt * 512:(qt + 1) * 512],
                                start=True, stop=True)
                        etw = etp.tile([128, 1024], BF16)
                        with nc.allow_low_precision(reason="bf16 probs"):
                            nc.scalar.activation(etw[:], psw[:], EXP)
                        return etw

                    def full_ctx(kb2, etw):
                        for s in (0, 1):
                            kb = kb2 + s
                            for qj in range(4):
                                nc.tensor.matmul(
                                    pctx[:, qj * 65:qj * 65 + 65],
                                    etw[:, s * 512 + qj * 128:
                                           s * 512 + (qj + 1) * 128],
                                    sb_v[:, kb * VW + h * 65:
                                            kb * VW + h * 65 + 65],
                                    start=(kb == 0 and qj == 0), stop=False,
                                    skip_group_check=True)

                    def diag_score(j):
                        kb = fulls + j
                        lo = j * 128
                        psc = gen_tile()
                        nc.tensor.matmul(
                            psc[:, lo:],
                            sb_k[pr][off:off + 64, kb * 128:(kb + 1) * 128],
                            sb_q[pr][off:off + 64,
                                     qt * 512 + lo:(qt + 1) * 512],
                            start=True, stop=True)
                        etd = etp.tile([128, 1024], BF16)
                        with nc.allow_low_precision(reason="bf16 probs"):
                            nc.scalar.activation(etd[:, lo:512], psc[:, lo:],
                                                 EXP)
                        with nc.allow_low_precision(reason="bf16 mask"):
                            nc.vector.tensor_mul(etd[:, lo:lo + 128],
                                                 etd[:, lo:lo + 128],
                                                 sb_mask[:])
                        return etd

                    def diag_ctx(j, etd):
                        kb = fulls + j
                        for qj in range(j, 4):
                            qc = 4 * qt + qj
                            nc.tensor.matmul(
                                pctx[:, qj * 65:qj * 65 + 65],
                                etd[:, qj * 128:(qj + 1) * 128],
                                sb_v[:, kb * VW + h * 65:
                                        kb * VW + h * 65 + 65],
                                start=(kb == 0 and qj == 0),
                                stop=(kb == qc),
                                skip_group_check=True)
                            if kb == qc:
                                rcp = rcpp.tile([128, 1], F32)
                                nc.vector.reciprocal(
                                    rcp[:], pctx[:, qj * 65 + 64:qj * 65 + 65])
                                with nc.allow_low_precision(reason="bf16 ctx"):
                                    nc.vector.tensor_scalar(
                                        ctxts[qj][:, h * 64:(h + 1) * 64],
                                        pctx[:, qj * 65:qj * 65 + 64],
                                        rcp[:], None, AluOpType.mult)

                    # one-step software pipeline: scores/exp for stage n+1
                    # are emitted before the ctx matmuls of stage n
                    stages = [(lambda kb2=kb2: full_score(kb2),
                               lambda et, kb2=kb2: full_ctx(kb2, et))
                              for kb2 in range(0, fulls, 2)]
                    stages += [(lambda j=j: diag_score(j),
                                lambda et, j=j: diag_ctx(j, et))
                               for j in range(4)]
                    prev = None
                    for sc, cx in stages:
                        et = sc()
                        if prev is not None:
                            prev[1](prev[0])
                        prev = (et, cx)
                        yield
                    prev[1](prev[0])
                    yield
                for qj in range(4):
                    finish_tb(qt, qj, ctxts[qj])
                    yield

            def weave(gens):
                # gens: list of generators or (generator, weight)
                gw = [(g, 1) if not isinstance(g, tuple) else g for g in gens]
                while gw:
                    alive = []
                    for g, w in gw:
                        done = False
                        for _ in range(w):
                            try:
                                next(g)
                            except StopIteration:
                                done = True
                                break
                        if not done:
                            alive.append((g, w))
                    gw = alive

            weave([proj_tt(0)])
            nc.sync.dma_start(xcol[1][:], xd[:, 3072:6144])
            nc.sync.dma_start(sb_wo[:], wod[:])
            nc.sync.dma_start(xcol[2][:], xd[:, 6144:9216])
            weave([(proj_tt(1), 1), (attn_qt(0), 4)])
            nc.sync.dma_start(xcol[3][:], xd[:, 9216:12288])
            weave([(proj_tt(2), 1), (attn_qt(1), 5)])
            weave([(proj_tt(3), 1), (attn_qt(2), 6)])
            weave([attn_qt(3)])

            if with_collective:
                nc.gpsimd.collective_compute(
                    "AllReduce", mybir.AluOpType.add,
                    replica_groups=[[0, 1], [2, 3], [4, 5], [6, 7]],
                    ins=[bounce_in.opt()], outs=[bounce_out.opt()])
                nc.sync.dma_start(out[:], bounce_out[:])
    nc.compile()
    return nc


def make_in_maps(x, w_q, w_k, w_v, w_o, token_positions):
    xn = np.asarray(x, np.float32)
    wqn = np.asarray(w_q, np.float32)
    wkn = np.asarray(w_k, np.float32)
    wvn = np.asarray(w_v, np.float32)
    won = np.asarray(w_o, np.float32)
    pos = np.asarray(token_positions).astype(np.float32)
    inv = THETA ** (-np.arange(32, dtype=np.float32) / 32.0)
    ang = inv[:, None] * pos[None, :]
    c32 = np.cos(ang).astype(np.float32)
    s32 = np.sin(ang).astype(np.float32)
    cblock = np.concatenate([c32[:16], c32[:16], c32[16:], c32[16:]], axis=0)
    sblock = np.concatenate([s32[:16], -s32[:16], s32[16:], -s32[16:]], axis=0)
    cosd = np.tile(cblock, (2, 1)).astype(BF)
    sind = np.tile(sblock, (2, 1)).astype(BF)
    maskd = (np.arange(128)[:, None] <= np.arange(128)[None, :]).astype(BF)
    eyed = np.eye(128, dtype=np.float32).astype(BF)
    perm_eo = np.r_[0:32:2, 1:32:2, 32:64:2, 33:64:2]
    in_maps = []
    for c in range(N_CORES):
        b, hg = c // 2, c % 2
        heads = hg * HPC + np.arange(HPC)
        rows_eo = (heads[:, None] * 64 + perm_eo[None, :]).reshape(-1)
        # x: xd[p, tt*3072 + ck*512 + s] = x[b, tt*512+s, ck*128+p]
        xd_ = (xn[b].reshape(4, 512, 6, 128).transpose(3, 0, 2, 1)
               .reshape(128, 4 * 3072)).astype(BF)
        # wq/wk: w*d[p, ck*384 + j] = w_perm[j, ck*128+p]
        wql = wqn[rows_eo] * 0.125
        wqd_ = (wql.reshape(384, 6, 128).transpose(2, 1, 0)
                .reshape(128, 2304)).astype(BF)
        wkl = wkn[rows_eo]
        wkd_ = (wkl.reshape(384, 6, 128).transpose(2, 1, 0)
                .reshape(128, 2304)).astype(BF)
        # wv: wvd[p, ck*390 + h*65 + jj] = wv[(hg*6+h)*64 + jj, ck*128+p]
        wvl = np.zeros((VW, D), np.float32)
        for h in range(HPC):
            g = hg * HPC + h
            wvl[h * 65:h * 65 + 64] = wvn[g * 64:(g + 1) * 64]
        wvd_ = (wvl.reshape(VW, 6, 128).transpose(2, 1, 0)
                .reshape(128, 6 * VW)).astype(BF)
        # wo: wod[p, ci*768 + od] = w_o[od, hg*384 + ci*128 + p]
        wol = won[:, hg * 384:(hg + 1) * 384]
        wod_ = (wol.T.reshape(3, 128, 768).transpose(1, 0, 2)
                .reshape(128, 2304)).astype(BF)
        in_maps.append({
            "xd": xd_, "wqd": wqd_, "wkd": wkd_, "wvd": wvd_, "wod": wod_,
            "cos": cosd, "sin": sind, "mask": maskd, "eye": eyed,
        })
    return in_maps


def kernel(x, w_q, w_k, w_v, w_o, token_positions):
    global _NC
    if _NC is None:
        _NC = build_nc()
    in_maps = make_in_maps(x, w_q, w_k, w_v, w_o, token_positions)
    res = run_bass_kernel_spmd(_NC, in_maps, core_ids=list(range(N_CORES)))
    return np.stack([res.results[2 * b]["out"] for b in range(B)], axis=0)


# revision 79
# speedup vs baseline: 1.0018x; 1.0018x over previous
import numpy as np
import ml_dtypes
import concourse.bass as bass
import concourse.mybir as mybir
import concourse.tile as tile
from concourse import bacc
from concourse.bass_utils import run_bass_kernel_spmd
from concourse.alu_op_type import AluOpType

B, S, D = 4, 2048, 768
HPC = 6            # heads per core
PAIRS = 3
THETA = 10000.0
N_CORES = 8
F32 = mybir.dt.float32
BF16 = mybir.dt.bfloat16
BF = ml_dtypes.bfloat16
VW = HPC * 65      # 390: per-tb V tile width (6 heads x (64 dims + ones col))
EXP = mybir.ActivationFunctionType.Exp

_NC = None


def build_nc(with_collective=True):
    nc = bacc.Bacc("TRN2", target_bir_lowering=False, debug=False,
                   num_devices=N_CORES)
    xd = nc.dram_tensor("xd", [128, 4 * 3072], BF16, kind="ExternalInput")
    wqd = nc.dram_tensor("wqd", [128, 2304], BF16, kind="ExternalInput")
    wkd = nc.dram_tensor("wkd", [128, 2304], BF16, kind="ExternalInput")
    wvd = nc.dram_tensor("wvd", [128, 6 * VW], BF16, kind="ExternalInput")
    wod = nc.dram_tensor("wod", [128, 2304], BF16, kind="ExternalInput")
    cosd = nc.dram_tensor("cos", [128, S], BF16, kind="ExternalInput")
    sind = nc.dram_tensor("sin", [128, S], BF16, kind="ExternalInput")
    maskd = nc.dram_tensor("mask", [128, 128], BF16, kind="ExternalInput")
    eyed = nc.dram_tensor("eye", [128, 128], BF16, kind="ExternalInput")
    out = nc.dram_tensor("out", [S, D], F32, kind="ExternalOutput")

    with tile.TileContext(nc) as tc:
        with tc.tile_pool(name="persist", bufs=1) as pp, \
             tc.tile_pool(name="dram", bufs=1, space="DRAM") as dpool, \
             tc.tile_pool(name="uhp", bufs=3) as uhp, \
             tc.tile_pool(name="swp", bufs=3) as swp, \
             tc.tile_pool(name="etp", bufs=13) as etp, \
             tc.tile_pool(name="ctxtp", bufs=10) as ctxtp, \
             tc.tile_pool(name="rcpp", bufs=6) as rcpp, \
             tc.tile_pool(name="obp", bufs=3) as obp, \
             tc.tile_pool(name="pgen", bufs=3, space="PSUM") as pgen, \
             tc.tile_pool(name="pwide", bufs=2, space="PSUM") as pwide, \
             tc.tile_pool(name="pctx", bufs=1, space="PSUM") as pctxp:

            sb_q = [pp.tile([128, S], BF16, name=f"sb_q{i}") for i in range(PAIRS)]
            sb_k = [pp.tile([128, S], BF16, name=f"sb_k{i}") for i in range(PAIRS)]
            sb_v = pp.tile([128, 16 * VW], BF16)
            sb_ctx = [pp.tile([128, S], BF16, name=f"sb_ctx{i}") for i in range(PAIRS)]
            xcol = [pp.tile([128, 3072], BF16, name=f"xcol{i}") for i in range(4)]
            sb_wq = pp.tile([128, 2304], BF16)
            sb_wk = pp.tile([128, 2304], BF16)
            sb_wv = pp.tile([128, 6 * VW], BF16)
            sb_wo = pp.tile([128, 2304], BF16)
            sb_cos = pp.tile([128, S], BF16)
            sb_sin = pp.tile([128, S], BF16)
            sb_mask = pp.tile([128, 128], BF16)
            sb_eye = pp.tile([128, 128], BF16)
            bounce_in = dpool.tile([S, D], F32)
            bounce_out = dpool.tile([S, D], F32)

            def gen_tile():
                return pgen.tile([128, 512], F32, name="pgen_t")

            def wide_tile():
                return pwide.tile([128, 1024], F32, name="pw_t")

            # input loads, most-urgent first
            nc.sync.dma_start(xcol[0][:], xd[:, 0:3072])
            nc.sync.dma_start(sb_wk[:], wkd[:])
            nc.sync.dma_start(sb_cos[:, 0:512], cosd[:, 0:512])
            nc.sync.dma_start(sb_sin[:, 0:512], sind[:, 0:512])
            nc.sync.dma_start(sb_wq[:], wqd[:])
            nc.sync.dma_start(sb_wv[:], wvd[:])
            nc.sync.dma_start(sb_cos[:, 512:2048], cosd[:, 512:2048])
            nc.sync.dma_start(sb_sin[:, 512:2048], sind[:, 512:2048])
            nc.sync.dma_start(sb_mask[:], maskd[:])
            nc.sync.dma_start(sb_eye[:], eyed[:])
            # warm up the PE p-state before real work arrives
            warm = pp.tile([128, 512], BF16)
            nc.vector.memset(warm[:], 1.0)
            wt = gen_tile()
            for i in range(7):
                nc.tensor.matmul(wt[:], warm[:, 0:128], warm[:],
                                 start=(i == 0), stop=(i == 6))
            with nc.allow_low_precision(reason="warmup drain"):
                nc.vector.tensor_copy(warm[:], wt[:])

            def proj_tt(tt):
                """QKV projection + RoPE for token block tt (512 tokens).

                K pairs first, swap+add per half so attention unblocks early;
                V chains interleaved to keep PE busy while DVE drains rope.
                """
                xc = xcol[tt]
                csl = sb_cos[:, tt * 512:(tt + 1) * 512]
                ssl = sb_sin[:, tt * 512:(tt + 1) * 512]
                uh = uhp.tile([128, 3072], BF16)
                swf = swp.tile([128, 3072], BF16)

                def qk_pair(wi, wsb, dst, pr):
                    if tt == 0:
                        pcw = wide_tile()
                        pc = pcw[:, 0:512]
                    else:
                        pc = gen_tile()
                    for ck in range(6):
                        nc.tensor.matmul(
                            pc[:],
                            wsb[:, ck * 384 + pr * 128:
                                   ck * 384 + (pr + 1) * 128],
                            xc[:, ck * 512:(ck + 1) * 512],
                            start=(ck == 0), stop=(ck == 5))
                    with nc.allow_low_precision(reason="bf16 qk"):
                        nc.vector.tensor_mul(
                            dst[pr][:, tt * 512:(tt + 1) * 512], pc[:], csl)
                        nc.vector.tensor_mul(
                            uh[:, (wi * 3 + pr) * 512:(wi * 3 + pr + 1) * 512],
                            pc[:], ssl)

                def v_block(tj):
                    tb = tt * 4 + tj
                    pvt = gen_tile()
                    for ck in range(6):
                        nc.tensor.matmul(
                            pvt[:, 0:VW],
                            xc[:, ck * 512 + tj * 128:
                                  ck * 512 + tj * 128 + 128],
                            sb_wv[:, ck * VW:(ck + 1) * VW],
                            start=(ck == 0), stop=(ck == 5))
                    with nc.allow_low_precision(reason="bf16 v"):
                        nc.scalar.copy(sb_v[:, tb * VW:(tb + 1) * VW],
                                       pvt[:, 0:VW])
                    nc.vector.memset(sb_v[:, tb * VW + 64:(tb + 1) * VW:65], 1.0)

                def swap_add(wi, dst, pr):
                    # rope pair swap: within each 32-partition block the
                    # even/odd halves are interleaved at 16, so the swap is
                    # an in-block shuffle i^16 on the DVE crossbar
                    sl = slice((wi * 3 + pr) * 512, (wi * 3 + pr + 1) * 512)
                    nc.vector.stream_shuffle(
                        swf[:, sl], uh[:, sl], [i ^ 16 for i in range(32)])
                    d = dst[pr][:, tt * 512:(tt + 1) * 512]
                    # pair 0 gates the next q-block's first score: keep its
                    # add on the low-latency DVE; offload the rest to Pool
                    aeng = nc.vector if pr == 0 else nc.gpsimd
                    with nc.allow_low_precision(reason="bf16 qk add"):
                        aeng.tensor_add(d, d, swf[:, sl])

                for pr in range(PAIRS):
                    qk_pair(0, sb_wk, sb_k, pr)
                    swap_add(0, sb_k, pr)
                    yield
                    qk_pair(1, sb_wq, sb_q, pr)
                    swap_add(1, sb_q, pr)
                    yield
                for tj in range(4):
                    v_block(tj)
                    yield

            def finish_tb(qt, qj, ctxt):
                """Transpose ctx to [hd, tok], output projection, store."""
                tb = qt * 4 + qj
                for c in range(PAIRS):
                    ptr = gen_tile()
                    nc.tensor.matmul(
                        ptr[:, 0:128],
                        ctxt[:, c * 128:(c + 1) * 128],
                        sb_eye[:],
                        start=True, stop=True)
                    with nc.allow_low_precision(reason="bf16 ctxT"):
                        if qt == 3 and c % 2 == 0:
                            nc.scalar.copy(
                                sb_ctx[c][:, tb * 128:(tb + 1) * 128],
                                ptr[:, 0:128])
                        else:
                            nc.vector.tensor_copy(
                                sb_ctx[c][:, tb * 128:(tb + 1) * 128],
                                ptr[:, 0:128])
                ob = obp.tile([128, D], F32)
                for nn in range(2):
                    pot = gen_tile()
                    for ci in range(PAIRS):
                        nc.tensor.matmul(
                            pot[:, 0:384],
                            sb_ctx[ci][:, tb * 128:(tb + 1) * 128],
                            sb_wo[:, ci * 768 + nn * 384:
                                     ci * 768 + nn * 384 + 384],
                            start=(ci == 0), stop=(ci == 2))
                    if qt == 3 and nn == 0:
                        nc.scalar.copy(ob[:, nn * 384:(nn + 1) * 384],
                                       pot[:, 0:384])
                    else:
                        nc.vector.tensor_copy(ob[:, nn * 384:(nn + 1) * 384],
                                              pot[:, 0:384])
                if with_collective:
                    nc.gpsimd.dma_start(bounce_in[tb * 128:(tb + 1) * 128, :],
                                        ob[:])
                else:
                    nc.sync.dma_start(out[tb * 128:(tb + 1) * 128, :], ob[:])

            def attn_qt(qt):
                """Full causal attention for q-block qt (512 queries), all
                6 heads, then output projection for its 4 token blocks."""
                nkb = 4 * qt + 4
                ctxts = [ctxtp.tile([128, 384], BF16, name="ctq")
                         for _ in range(4)]
                for h in range(HPC):
                    pr, off = h // 2, (h % 2) * 64
                    pctx = pctxp.tile([128, 260], F32, name="pctx_t")
                    fulls = 4 * qt

                    def full_score(kb2):
                        psw = wide_tile()
                        for s in (0, 1):
                            kb = kb2 + s
                            nc.tensor.matmul(
                                psw[:, s * 512:(s + 1) * 512],
                                sb_k[pr][off:off + 64, kb * 128:(kb + 1) * 128],
                                sb_q[pr][off:off + 64, qt * 512:(qt + 1) * 512],
                                start=True, stop=True)
                        etw = etp.tile([128, 1024], BF16)
                        with nc.allow_low_precision(reason="bf16 probs"):
                            nc.scalar.activation(etw[:], psw[:], EXP)
                        return etw

                    def full_ctx(kb2, etw):
                        for s in (0, 1):
                            kb = kb2 + s
                            for qj in range(4):
                                nc.tensor.matmul(
                                    pctx[:, qj * 65:qj * 65 + 65],
                                    etw[:, s * 512 + qj * 128:
                                           s * 512 + (qj + 1) * 128],
                                    sb_v[:, kb * VW + h * 65:
                                            kb * VW + h * 65 + 65],
                                    start=(kb == 0 and qj == 0), stop=False,
                                    skip_group_check=True)

                    def diag_score(j):
                        kb = fulls + j
                        lo = j * 128
                        psc = gen_tile()
                        nc.tensor.matmul(
                            psc[:, lo:],
                            sb_k[pr][off:off + 64, kb * 128:(kb + 1) * 128],
                            sb_q[pr][off:off + 64,
                                     qt * 512 + lo:(qt + 1) * 512],
                            start=True, stop=True)
                        etd = etp.tile([128, 1024], BF16)
                        with nc.allow_low_precision(reason="bf16 probs"):
                            nc.scalar.activation(etd[:, lo:512], psc[:, lo:],
                                                 EXP)
                        with nc.allow_low_precision(reason="bf16 mask"):
                            nc.vector.tensor_mul(etd[:, lo:lo + 128],
                                                 etd[:, lo:lo + 128],
                                                 sb_mask[:])
                        return etd

                    def diag_ctx(j, etd):
                        kb = fulls + j
                        for qj in range(j, 4):
                            qc = 4 * qt + qj
                            nc.tensor.matmul(
                                pctx[:, qj * 65:qj * 65 + 65],
                                etd[:, qj * 128:(qj + 1) * 128],
                                sb_v[:, kb * VW + h * 65:
                                        kb * VW + h * 65 + 65],
                                start=(kb == 0 and qj == 0),
                                stop=(kb == qc),
                                skip_group_check=True)
                            if kb == qc:
                                rcp = rcpp.tile([128, 1], F32)
                                nc.vector.reciprocal(
                                    rcp[:], pctx[:, qj * 65 + 64:qj * 65 + 65])
                                with nc.allow_low_precision(reason="bf16 ctx"):
                                    nc.vector.tensor_scalar(
                                        ctxts[qj][:, h * 64:(h + 1) * 64],
                                        pctx[:, qj * 65:qj * 65 + 64],
                                        rcp[:], None, AluOpType.mult)

                    # one-step software pipeline: scores/exp for stage n+1
                    # are emitted before the ctx matmuls of stage n
                    stages = [(lambda kb2=kb2: full_score(kb2),
                               lambda et, kb2=kb2: full_ctx(kb2, et))
                              for kb2 in range(0, fulls, 2)]
                    stages += [(lambda j=j: diag_score(j),
                                lambda et, j=j: diag_ctx(j, et))
                               for j in range(4)]
                    prev = None
                    for sc, cx in stages:
                        et = sc()
                        if prev is not None:
                            prev[1](prev[0])
                        prev = (et, cx)
                        yield
                    prev[1](prev[0])
                    yield
                for qj in range(4):
                    finish_tb(qt, qj, ctxts[qj])
                    yield

            def weave(gens):
                # gens: list of generators or (generator, weight)
                gw = [(g, 1) if not isinstance(g, tuple) else g for g in gens]
                while gw:
                    alive = []
                    for g, w in gw:
                        done = False
                        for _ in range(w):
                            try:
                                next(g)
                            except StopIteration:
                                done = True
                                break
                        if not done:
                            alive.append((g, w))
                    gw = alive

            weave([proj_tt(0)])
            nc.sync.dma_start(xcol[1][:], xd[:, 3072:6144])
            nc.sync.dma_start(sb_wo[:], wod[:])
            nc.sync.dma_start(xcol[2][:], xd[:, 6144:9216])
            weave([(proj_tt(1), 1), (attn_qt(0), 4)])
            nc.sync.dma_start(xcol[3][:], xd[:, 9216:12288])
            weave([(proj_tt(2), 1), (attn_qt(1), 5)])
            weave([(proj_tt(3), 1), (attn_qt(2), 6)])
            weave([attn_qt(3)])

            if with_collective:
                nc.gpsimd.collective_compute(
                    "AllReduce", mybir.AluOpType.add,
                    replica_groups=[[0, 1], [2, 3], [4, 5], [6, 7]],
                    ins=[bounce_in.opt()], outs=[bounce_out.opt()])
                nc.sync.dma_start(out[:], bounce_out[:])
    nc.compile()
    return nc


def make_in_maps(x, w_q, w_k, w_v, w_o, token_positions):
    xn = np.asarray(x, np.float32)
    wqn = np.asarray(w_q, np.float32)
    wkn = np.asarray(w_k, np.float32)
    wvn = np.asarray(w_v, np.float32)
    won = np.asarray(w_o, np.float32)
    pos = np.asarray(token_positions).astype(np.float32)
    inv = THETA ** (-np.arange(32, dtype=np.float32) / 32.0)
    ang = inv[:, None] * pos[None, :]
    c32 = np.cos(ang).astype(np.float32)
    s32 = np.sin(ang).astype(np.float32)
    cblock = np.concatenate([c32[:16], c32[:16], c32[16:], c32[16:]], axis=0)
    sblock = np.concatenate([s32[:16], -s32[:16], s32[16:], -s32[16:]], axis=0)
    cosd = np.tile(cblock, (2, 1)).astype(BF)
    sind = np.tile(sblock, (2, 1)).astype(BF)
    maskd = (np.arange(128)[:, None] <= np.arange(128)[None, :]).astype(BF)
    eyed = np.eye(128, dtype=np.float32).astype(BF)
    perm_eo = np.r_[0:32:2, 1:32:2, 32:64:2, 33:64:2]
    in_maps = []
    for c in range(N_CORES):
        b, hg = c // 2, c % 2
        heads = hg * HPC + np.arange(HPC)
        rows_eo = (heads[:, None] * 64 + perm_eo[None, :]).reshape(-1)
        # x: xd[p, tt*3072 + ck*512 + s] = x[b, tt*512+s, ck*128+p]
        xd_ = (xn[b].reshape(4, 512, 6, 128).transpose(3, 0, 2, 1)
               .reshape(128, 4 * 3072)).astype(BF)
        # wq/wk: w*d[p, ck*384 + j] = w_perm[j, ck*128+p]
        wql = wqn[rows_eo] * 0.125
        wqd_ = (wql.reshape(384, 6, 128).transpose(2, 1, 0)
                .reshape(128, 2304)).astype(BF)
        wkl = wkn[rows_eo]
        wkd_ = (wkl.reshape(384, 6, 128).transpose(2, 1, 0)
                .reshape(128, 2304)).astype(BF)
        # wv: wvd[p, ck*390 + h*65 + jj] = wv[(hg*6+h)*64 + jj, ck*128+p]
        wvl = np.zeros((VW, D), np.float32)
        for h in range(HPC):
            g = hg * HPC + h
            wvl[h * 65:h * 65 + 64] = wvn[g * 64:(g + 1) * 64]
        wvd_ = (wvl.reshape(VW, 6, 128).transpose(2, 1, 0)
                .reshape(128, 6 * VW)).astype(BF)
        # wo: wod[p, ci*768 + od] = w_o[od, hg*384 + ci*128 + p]
        wol = won[:, hg * 384:(hg + 1) * 384]
        wod_ = (wol.T.reshape(3, 128, 768).transpose(1, 0, 2)
                .reshape(128, 2304)).astype(BF)
        in_maps.append({
            "xd": xd_, "wqd": wqd_, "wkd": wkd_, "wvd": wvd_, "wod": wod_,
            "cos": cosd, "sin": sind, "mask": maskd, "eye": eyed,
        })
    return in_maps


def kernel(x, w_q, w_k, w_v, w_o, token_positions):
    global _NC
    if _NC is None:
        _NC = build_nc()
    in_maps = make_in_maps(x, w_q, w_k, w_v, w_o, token_positions)
    res = run_bass_kernel_spmd(_NC, in_maps, core_ids=list(range(N_CORES)))
    return np.stack([res.results[2 * b]["out"] for b in range(B)], axis=0)
